# revision 41
# baseline (speedup 1.0000x reference)
"""CAML kernel for Trainium2: embed-gather -> conv1d(tanh) -> label attention -> per-class dot.

Sharding: data-parallel over batch, one batch element per NeuronCore (B=8, 8 cores).
Each core runs an identical Bass program on its own slice.

Per-core layout (hardcoded for B=8,S=2048,V=32000,D=512,K=256,T=9,C=4096):
  - embed gather via SWDGE dma_gather(transpose=True) from an fp8 table (8x
    scaled) with an appended all-zero row (index 32000) for conv same-padding;
    4 overlapping 640-token chunks on 4 SWDGE queues. fp8 transpose works at
    16-bit granularity so adjacent d-pairs interleave: the gathered tile
    [128, 4, 640] viewed as [128, j:2, t:640, e:2] holds d = 2*(j*128+p)+e;
    conv weights are host-permuted to match.
  - warm-up junk matmuls keep the PE HAM un-throttled while the gather runs.
  - conv as 9 shifted fp8 DoubleRow matmuls per (d-half, k-chunk) accumulated
    in PSUM; one tanh(+bias) evacuation on ScalarE -> xcT [k, s] fp8.
  - xcT -> xa8 [s, 257] fp8 via PE transposes (fp8 identity), 4 transposes
    batched per PSUM bank, one DVE evacuation each; col 256 memset to 1.0
    (softmax denominator).
  - label attention, 512 classes/block, software-pipelined: per si-pair,
    scoresT [tok, c] = xcT.T @ U_wT into PSUM; exp split between ScalarE
    (ACT Exp -> fp8) and DVE (Schraudolph bit-trick: fp8e4m3 bits =
    round(score*log2e*8 + 56) computed as one tensor_scalar into int8,
    bitcast to fp8 - equivalent precision to rounding exp to fp8);
    mu[c, 257] = e8.T @ xa8 accumulated over s with a 1-iteration lag so
    the PE never waits on exp.
  - mu -> SBUF via 2 DVE copies per block; dots = sum_k mu*final_w on GpSimd
    (scalar_tensor_tensor w/ accum), y = dots/dens + fb on GpSimd.
"""

import numpy as np
import ml_dtypes

import concourse.bacc as bacc
import concourse.mybir as mybir
import concourse.tile as tile
from concourse import library_config
from concourse.bass_utils import run_bass_kernel_spmd

F32 = mybir.dt.float32
BF16 = mybir.dt.bfloat16
F8 = mybir.dt.float8e4
I16 = mybir.dt.int16
I8 = mybir.dt.int8
AF = mybir.ActivationFunctionType
ALU = mybir.AluOpType
DR = mybir.MatmulPerfMode.DoubleRow

B, S, VOCAB, D, NK, KT, C = 8, 2048, 32000, 512, 256, 9, 4096
PAD = 4
NIDX = 2176            # 4 pad + 2048 + 4 pad + 120 dummy; multiple of 128
ZROW = VOCAB           # index of the appended all-zero embed row
NSI = S // 128         # 16 sequence chunks
NCB = C // 512         # 8 class blocks
NCJ = C // 128         # 32 class chunks
DC = D // 128          # 4 d chunks
KC = NK // 128         # 2 k chunks
LOG2E = 1.4426950408889634
EXPC = 56.0            # fp8e4m3 exponent-bias term: 8 * bias(7)
GATHER_FP8 = True      # bisect flag: fp8 table gather vs bf16 gather + DVE cast
EXP_TRICK = True       # bisect flag: DVE int8 Schraudolph exp vs all-scalar ACT
WARMUP_N = 24         # bisect flag: PE warm-up junk matmuls during gather


def build_nc(debug=False):
    nc = bacc.Bacc("TRN2", target_bir_lowering=False, debug=debug)

    # table/convw/uw are pre-scaled by 8 on the host so fp8(e4m3) values sit in
    # the normal range; the 1/64 (conv) and 1/8 (scores) descale happens in the
    # tanh ACT `scale` and the exp constants.
    p_table = nc.declare_dram_parameter(
        "table", [VOCAB + 1, D], F8 if GATHER_FP8 else BF16, isOutput=False)
    p_idxs = nc.declare_dram_parameter("idxs", [128, 160], I16, isOutput=False)
    p_w = nc.declare_dram_parameter("convw", [128, 36, 2, 128], F8, isOutput=False)
    p_u = nc.declare_dram_parameter("uw", [128, KC, C], F8, isOutput=False)
    p_fw = nc.declare_dram_parameter("fw", [128, NCJ, NK], F32, isOutput=False)
    p_fb = nc.declare_dram_parameter("fb", [128, NCJ], F32, isOutput=False)
    p_cb = nc.declare_dram_parameter("cb", [128, KC], F32, isOutput=False)
    p_id = nc.declare_dram_parameter("ident", [128, 128], BF16, isOutput=False)
    p_ones = nc.declare_dram_parameter("ones", [128, 1], BF16, isOutput=False)
    p_out = nc.declare_dram_parameter("out", [128, NCJ], F32, isOutput=True)

    with tile.TileContext(nc) as tc:
        with (
            tc.tile_pool(name="consts", bufs=1) as cp,
            tc.tile_pool(name="acts", bufs=1) as ap,
            tc.tile_pool(name="exps", bufs=3) as ep,
            tc.tile_pool(name="musb", bufs=2) as mb,
            tc.tile_pool(name="scratch", bufs=2) as scp,
        ):
            idx_sb = cp.tile([128, 160], I16)
            w_sb = cp.tile([128, 36, 2, 128], F8)
            u_sb = cp.tile([128, KC, C], F8)
            fw_sb = cp.tile([128, NCJ, NK], F32)
            fb_sb = cp.tile([128, NCJ], F32)
            cb_sb = cp.tile([128, KC], F32)
            id_sb = cp.tile([128, 128], BF16)
            ones_sb = cp.tile([128, 1], BF16)

            gdt = F8 if GATHER_FP8 else BF16
            xts = [ap.tile([128, DC, 640], gdt, name=f"xt{i}", tag=f"xt{i}") for i in range(4)]
            if not GATHER_FP8:
                xt8s = [ap.tile([128, DC, 640], F8, name=f"xt8{i}", tag=f"xt8{i}") for i in range(4)]
            xcT = ap.tile([128, KC, S], F8)           # conv output, k-major fp8
            xcTb = ap.tile([128, KC, S], BF16)        # same, bf16 (transpose path)
            # s-major features + ones col: xa8[tok, si, k]; k=256 is ones.
            xa8 = ap.tile([128, NSI, 272], F8)
            dots = ap.tile([128, NCJ], F32)
            dens = ap.tile([128, NCJ], F32)
            y_sb = ap.tile([128, NCJ], F32)

            # --- input DMAs -------------------------------------------------
            nc.gpsimd.load_library(library_config.mlp)
            nc.sync.dma_start(idx_sb[:, :], p_idxs[:, :])
            nc.sync.dma_start(w_sb[:, :, :, :], p_w[:, :, :, :])
            nc.sync.dma_start(cb_sb[:, :], p_cb[:, :])
            nc.sync.dma_start(id_sb[:, :], p_id[:, :])
            nc.sync.dma_start(ones_sb[:, :], p_ones[:, :])
            nc.sync.dma_start(u_sb[:, :, :], p_u[:, :, :])
            nc.sync.dma_start(fb_sb[:, :], p_fb[:, :])
            nc.sync.dma_start(fw_sb[:, :, :], p_fw[:, :, :])

            nidx_reg = nc.gpsimd.compute_val(640)
            for i in range(4):
                nc.gpsimd.dma_gather(
                    xts[i][:, :, :], p_table[:, :], idx_sb[:, i * 40:(i + 1) * 40],
                    640, nidx_reg, D, transpose=True, single_packet=False,
                )
                if not GATHER_FP8:
                    nc.vector.tensor_copy(xt8s[i][:, :, :], xts[i][:, :, :])

            if GATHER_FP8:
                # interleaved-pair views: xv[p,j,t,e] = embed[tok_t, 2*(j*128+p)+e]
                xvs = [
                    x[:, :, :].rearrange("p a t -> p (a t)").rearrange(
                        "p (j t e) -> p j t e", j=2, t=640, e=2)
                    for x in xts
                ]

            # --- conv1d + transposes ---------------------------------------
            # psum holds 64x the true conv (inputs 8x-scaled) -> tanh scale=1/64.
            with (
                tc.tile_pool(name="cps", bufs=4, space="PSUM") as cps,
                tc.tile_pool(name="tps", bufs=2, space="PSUM") as tps,
                tc.tile_pool(name="wps", bufs=1, space="PSUM") as wps,
            ):
                # PE warm-up: junk DR matmuls over already-landed weights keep
                # the HAM clock at 8/8 while the first gather chunk arrives.
                junk = wps.tile([128, 512], F32)
                wview = w_sb[:, 0:8, :, :].rearrange("p a h k -> p (a h k)").rearrange(
                    "p (h t) -> p h t", h=2, t=1024)
                for i in range(WARMUP_N):
                    nc.tensor.matmul(
                        junk[:, :], w_sb[:, i % 8, :, :], wview[:, :, 0:512],
                        start=True, stop=True, perf_mode=DR,
                    )

                def emit_transposes(sc):
                    # bf16 PE transposes of xcTb, 2 (kc) per si chunk batched
                    # into one PSUM tile, one DVE cast-copy into xa8.
                    for si in range(sc * 4, sc * 4 + 4):
                        tp = tps.tile([128, 2, 128], BF16, name=f"tp{si}", tag="tp")
                        for kc in range(KC):
                            nc.tensor.transpose(
                                tp[:, kc, :],
                                xcTb[:, kc, si * 128:(si + 1) * 128],
                                id_sb[:, :],
                            )
                        nc.vector.tensor_copy(
                            xa8[:, si, 0:256],
                            tp[:, :, :].rearrange("p c k -> p (c k)"),
                        )
                        nc.vector.tensor_copy(xa8[:, si, 256:257], ones_sb[:, :])

                for sc in range(4):
                    for kc in range(KC):
                        pt = cps.tile([128, 512], F32, name=f"cps_{sc}_{kc}", tag="cps")
                        it = 0
                        for c2 in range(2):
                            for t in range(KT):
                                rhs = (
                                    xvs[sc][:, c2, t:t + 512, :].rearrange(
                                        "p t e -> p e t")
                                    if GATHER_FP8 else
                                    xt8s[sc][:, 2 * c2:2 * c2 + 2, t:t + 512]
                                )
                                nc.tensor.matmul(
                                    pt[:, :],
                                    w_sb[:, (c2 * KT + t) * KC + kc, :, :],
                                    rhs,
                                    start=(it == 0),
                                    stop=(it == KT * 2 - 1),
                                    perf_mode=DR,
                                )
                                it += 1
                        nc.scalar.activation(
                            xcT[:, kc, sc * 512:(sc + 1) * 512],
                            pt[:, :],
                            AF.Tanh,
                            bias=cb_sb[:, kc:kc + 1],
                            scale=1.0 / 64.0,
                        )
                        nc.scalar.activation(
                            xcTb[:, kc, sc * 512:(sc + 1) * 512],
                            pt[:, :],
                            AF.Tanh,
                            bias=cb_sb[:, kc:kc + 1],
                            scale=1.0 / 64.0,
                        )
                    if sc >= 1:
                        emit_transposes(sc - 1)
                emit_transposes(3)

            # --- label attention, software-pipelined ------------------------
            # Steady state per si-pair (872ns of PE): scores x2, exp split
            # ScalarE (SPLA-wide ACT) / DVE (rest, bit-trick), mu x4 lagged one
            # iteration so the PE never waits on exp. Block tails: mu PSUM
            # drains right at the boundary (PSUM-reuse deadline) via 2 ScalarE
            # ACT-copies (cs01) + 2 DVE copies (cs23) into an SBUF staging
            # tile; the dots run later, spread as ONE batched [128,4,256]
            # mul + one batched reduce per block in DVE slack.
            with (
                tc.tile_pool(name="sps", bufs=2, space="PSUM") as sps,
                tc.tile_pool(name="mps", bufs=1, space="PSUM") as mps,  # 4 tags x 1 buf
            ):
                e8s = [None] * 65
                mss = [None] * 8
                mu_t = None

                def emit_dot(tcb, step):
                    # dots via DVE scalar_tensor_tensor with accumulate: one
                    # fused (mu*fw -> scratch, sum -> dots) op per class chunk
                    cj = tcb * 4 + step
                    ms = mss[tcb]
                    scr = scp.tile([128, NK], F32, name=f"scr{tcb}_{step}", tag="scr")
                    nc.vector.scalar_tensor_tensor(
                        scr[:, :], ms[:, step, 0:256], 1.0, fw_sb[:, cj, :],
                        ALU.mult, ALU.mult,
                        accum_out=dots[:, cj:cj + 1],
                    )
                    if step == 1:
                        nc.vector.tensor_copy(
                            dens[:, tcb * 4:tcb * 4 + 2], ms[:, :, 256:257])

                for idx in range(65):
                    cb, sj = divmod(idx, 8)
                    # 1) tensor work first: scores(idx) then lagged mu(idx-1)
                    if idx < 64:
                        sp = sps.tile([128, 1024], F32, name=f"sp{idx}", tag="sp")
                        for h in range(2):
                            si = 2 * sj + h
                            nc.tensor.matmul(
                                sp[:, h * 512:(h + 1) * 512],
                                xcT[:, :, si * 128:(si + 1) * 128],
                                u_sb[:, :, cb * 512:(cb + 1) * 512],
                                start=True, stop=True, perf_mode=DR,
                            )
                    if idx >= 1:
                        pcb, psj = divmod(idx - 1, 8)
                        if psj == 0:
                            mu_t = [
                                mps.tile([128, NK + 1], F32, name=f"mu{pcb}_{cs}", tag=f"mu{cs}")
                                for cs in range(4)
                            ]
                        pe8a, pe8b = e8s[idx - 1]
                        for cs in range(4):
                            lhs = (pe8a if cs < 2 else pe8b)
                            nc.tensor.matmul(
                                mu_t[cs][:, 0:257],
                                lhs[:, :, (cs % 2) * 128:(cs % 2 + 1) * 128],
                                xa8[:, 2 * psj:2 * psj + 2, 0:257],
                                start=(psj == 0), stop=(psj == 7),
                                perf_mode=DR,
                            )
                    # 2) drains/dots BEFORE this idx's exps so the DVE queue
                    # head never blocks ready work behind a waiting expB.
                    # tail for the block whose mu just got its stop matmul
                    # (the "mu" realloc for the next block is at idx%8==1, so
                    # the finished tile is always the current mu_t here).
                    if idx >= 8 and idx % 8 == 0:
                        tcb = idx // 8 - 1
                        ms = mb.tile([128, 2, 257], F32, name=f"ms{tcb}", tag="ms")
                        mss[tcb] = ms
                        # cs0/1: ScalarE drains to SBUF (dots spread later);
                        # cs2/3: fused dot directly from PSUM on DVE + dens.
                        nc.scalar.copy(ms[:, 0, :], mu_t[0][:, 0:257])
                        nc.scalar.copy(ms[:, 1, :], mu_t[1][:, 0:257])
                        for cs in (2, 3):
                            cj = tcb * 4 + cs
                            scr = scp.tile([128, NK], F32, name=f"scrd{tcb}_{cs}", tag="scr")
                            nc.vector.scalar_tensor_tensor(
                                scr[:, :], mu_t[cs][:, 0:256], 1.0, fw_sb[:, cj, :],
                                ALU.mult, ALU.mult,
                                accum_out=dots[:, cj:cj + 1],
                            )
                            nc.vector.tensor_copy(
                                dens[:, cj:cj + 1], mu_t[cs][:, 256:257])
                    if idx >= 1:
                        pcb, psj = divmod(idx - 1, 8)
                        # spread the previous block's dots into DVE slack
                        if 1 <= psj <= 2 and pcb >= 1:
                            emit_dot(pcb - 1, psj - 1)
                    # 3) exps for this idx
                    if idx < 64:
                        # two e8 tiles split by class half so the ScalarE and
                        # DVE exp halves have no shared-tile WAW serialization
                        # and each mu chunk waits only on its own exp engine.
                        e8a = ep.tile([128, 2, 256], F8, name=f"e8a_{idx}", tag="e8a")
                        e8b = ep.tile([128, 2, 256], F8, name=f"e8b_{idx}", tag="e8b")
                        e8s[idx] = (e8a, e8b)
                        spv = sp[:, :].rearrange("p (h c) -> p h c", h=2, c=512)
                        nc.scalar.activation(
                            e8a[:, :, :], spv[:, :, 0:256],
                            AF.Exp, scale=1.0 / 8.0,
                        )
                        if EXP_TRICK:
                            nc.vector.tensor_scalar(
                                e8b[:, :, :].bitcast(I8), spv[:, :, 256:512],
                                LOG2E, EXPC, ALU.mult, ALU.add,
                            )
                        else:
                            nc.scalar.activation(
                                e8b[:, :, :], spv[:, :, 256:512],
                                AF.Exp, scale=1.0 / 8.0,
                            )

                for step in range(2):
                    emit_dot(7, step)
                rcp = scp.tile([128, NCJ], F32, name="rcp", tag="rcp")
                nc.vector.reciprocal(rcp[:, :], dens[:, :])
                nc.vector.tensor_mul(y_sb[:, :], dots[:, :], rcp[:, :])
                nc.vector.tensor_add(y_sb[:, :], y_sb[:, :], fb_sb[:, :])

            nc.sync.dma_start(p_out[:, :], y_sb[:, :])

    nc.compile()
    return nc


def prep_shared(embed_table, conv_w, conv_b, U_w, final_w, final_b):
    """Host-side layout transforms shared by all cores (cast/scale/transpose only).

    table, conv_w, U_w are scaled by 8 so their fp8(e4m3) quantization happens
    in the normal range; the kernel descales via the tanh ACT `scale` (1/64)
    and the exp constants (1/8).
    """
    bf = ml_dtypes.bfloat16
    f8 = ml_dtypes.float8_e4m3
    tdt = f8 if GATHER_FP8 else bf
    table = np.zeros((VOCAB + 1, D), dtype=tdt)
    table[:VOCAB] = (np.asarray(embed_table, np.float32) * 8.0).astype(tdt)
    if GATHER_FP8:
        # gather layout: partition p, chunk j, byte e holds d = 2*(j*128+p)+e.
        # w_sb[p, (c2*KT+t)*KC+kc, h, ki] = 8*conv_w[kc*128+ki, 2*(c2*128+p)+h, t]
        cw = np.ascontiguousarray(np.asarray(conv_w, np.float32) * 8.0)
        cw = cw.transpose(1, 0, 2)
        cw = cw.reshape(2, 128, 2, KC, 128, KT)
        w_host = np.ascontiguousarray(
            cw.transpose(1, 0, 5, 3, 2, 4)
        ).reshape(128, 36, 2, 128).astype(f8)
    else:
        # d = c2*256 + h*128 + di (baseline gather layout)
        cw = np.ascontiguousarray(np.asarray(conv_w, np.float32) * 8.0).reshape(
            KC, 128, 2, 2, 128, KT)
        w_host = np.ascontiguousarray(
            cw.transpose(4, 2, 5, 0, 3, 1)).reshape(128, 36, 2, 128).astype(f8)
    # u_host[ki, h, c] = 8*U_w[c, h*128+ki]
    u_host = np.ascontiguousarray(
        (np.asarray(U_w, np.float32).T * 8.0).reshape(KC, 128, C).transpose(1, 0, 2)
    ).astype(f8)
    fw_host = np.ascontiguousarray(
        np.asarray(final_w, np.float32).reshape(NCJ, 128, NK).transpose(1, 0, 2)
    ).astype(np.float32)
    fb_host = np.ascontiguousarray(np.asarray(final_b, np.float32).reshape(NCJ, 128).T).astype(np.float32)
    cb_host = np.ascontiguousarray(np.asarray(conv_b, np.float32).reshape(KC, 128).T).astype(np.float32)
    ident = np.eye(128, dtype=bf)
    ones = np.ones((128, 1), dtype=bf)
    return {
        "table": table, "convw": w_host, "uw": u_host, "fw": fw_host,
        "fb": fb_host, "cb": cb_host, "ident": ident, "ones": ones,
    }


def prep_idxs(text_row):
    toks = np.full(NIDX, ZROW, dtype=np.int16)
    toks[PAD:PAD + S] = text_row.astype(np.int16)
    # 4 overlapping 640-token chunks (chunk i covers padded positions
    # [i*512, i*512+640)), each wrapped [16, 40], stacked along columns.
    cols = []
    for i in range(4):
        chunk = toks[i * 512:i * 512 + 640]
        cols.append(chunk.reshape(40, 16).T)      # [16, 40]
    lay = np.concatenate(cols, axis=1)            # [16, 160]
    return np.ascontiguousarray(np.tile(lay, (8, 1)))  # [128, 160]


_NC_CACHE = {}


def get_nc(debug=False):
    if debug not in _NC_CACHE:
        _NC_CACHE[debug] = build_nc(debug=debug)
    return _NC_CACHE[debug]


def make_in_maps(text, shared):
    return [dict(shared, idxs=prep_idxs(np.asarray(text)[i])) for i in range(B)]


def kernel(text, embed_table, conv_w, conv_b, U_w, final_w, final_b, _trace=False):
    text = np.asarray(text)
    shared = prep_shared(
        np.asarray(embed_table), np.asarray(conv_w), np.asarray(conv_b),
        np.asarray(U_w), np.asarray(final_w), np.asarray(final_b),
    )
    in_maps = make_in_maps(text, shared)
    nc = get_nc()
    res = run_bass_kernel_spmd(nc, in_maps, list(range(B)), trace=_trace)
    out = np.stack([
        np.asarray(res.results[i]["out"]).T.reshape(C) for i in range(B)
    ]).astype(np.float32)
    if _trace:
        kernel.last_exec_time_ns = res.exec_time_ns
        kernel.last_results = res
    return out
